# revision 6
# baseline (speedup 1.0000x reference)
"""DGCNN (4x GCNConv + sort-pool + MLP) on 8 trn2 NeuronCores.

Strategy: graph-parallel sharding (ranks 0-3: 13 graphs, 4-7: 12).
Interleaved slot layout (slot = per-graph degree rank * 13 + graph) so
each 128-dst round holds a narrow degree band across all graphs.
Per layer: the full u = dinv*h table (f32, node-major, with a 128-row
zero gap between pieces) is AllGather'd to every core; per-edge source
rows are fetched with batched SWDGE dma_gather (one instruction per
~48*128 rows instead of one indirect DMA per 128 rows).  dma_gather
indices are int16, so the 53376-row table is covered by three
overlapping 32768-row windows; a per-round LP + greedy assignment
balances edges across windows to keep padding ~25%.  Self-loops are
folded into the edge list.  Per round: one strided tensor_reduce per
window + combine + dinv scale on DVE, PE transpose, feature transform
+ tanh per 4 rounds with inline staging of the next layer's table.
Sort-pool via max8/max_index/match_replace, pooled rows extracted with
ap_gather, classifier on PE.
"""
import os
import numpy as np

N = 50000
G = 100
NPG = 500
E = 800000
F = 64
K_TOP = 15
CAT = 193
NCORES = 8
SHARD = 6656
NROUND = SHARD // 128  # 52
BN_EPS = 1e-5
NW = 3                 # gather windows
NTOT2 = 53376          # 8*6656 + 128-row zero gap
WIN = 32768
WBASE = [0, 10304, 20608]
GAPROW = 26624         # first zero-gap row (inside all 3 windows)
PBASE = [0, 26752, 40064]
CH = 48                # max gather columns per window per chunk

GRAPHS_PER_CORE = [13, 13, 13, 13, 12, 12, 12, 12]
GSTART = np.concatenate([[0], np.cumsum(GRAPHS_PER_CORE)])

_CACHE = {}


def _prep(x, edge_index):
    """Host-side sharding/index preprocessing. Pure numpy."""
    src = edge_index[0].astype(np.int64)
    dst = edge_index[1].astype(np.int64)
    # self-loops folded into the edge list
    src = np.concatenate([src, np.arange(N)])
    dst = np.concatenate([dst, np.arange(N)])

    deg = np.bincount(dst, minlength=N).astype(np.float32)  # includes self
    dinv = deg ** np.float32(-0.5)
    indeg = np.bincount(dst, minlength=N).astype(np.int64)

    node_graph = np.arange(N) // NPG
    node_rank = np.searchsorted(GSTART, node_graph, side="right") - 1
    slot_of = np.zeros(N, np.int64)
    for g in range(G):
        lo = g * NPG
        o = np.argsort(-indeg[lo:lo + NPG], kind="stable")
        r = np.empty(NPG, np.int64)
        r[o] = np.arange(NPG)
        gl = g - GSTART[np.searchsorted(GSTART, g, side="right") - 1]
        slot_of[lo:lo + NPG] = r * 13 + gl

    piece = ((slot_of >= 3328).astype(np.int64)
             + (slot_of >= 4992).astype(np.int64))
    pbase = np.array(PBASE)
    prows = np.array([3328, 1664, 1664])
    plo = np.array([0, 3328, 4992])
    pidx = pbase[piece] + node_rank * prows[piece] + (slot_of - plo[piece])

    rows = pidx[src]
    d_slot = slot_of[dst]
    e_rank = node_rank[dst]

    # region by source row: 0=[0,10304) W0-only, 1=[10304,20608) W01,
    # 2=[20608,32768) W012, 3=[32768,43072) W12, 4=[43072,..) W2-only
    region = np.digitize(rows, [10304, 20608, 32768, 43072])

    key = e_rank * SHARD + d_slot
    cnt_r = np.stack([np.bincount(key[region == r], minlength=8 * SHARD)
                      for r in range(5)])
    tot = cnt_r.sum(axis=0)

    def rmax(a):
        return a.reshape(8, NROUND, 128).max(axis=(0, 2))

    a1 = rmax(cnt_r[0])
    a12 = rmax(cnt_r[0] + cnt_r[1])
    a5 = rmax(cnt_r[4])
    a45 = rmax(cnt_r[3] + cnt_r[4])
    a15 = rmax(cnt_r[0] + cnt_r[4])
    aall = rmax(tot)

    # per-round exact LP (small search) for window call counts T0+T1+T2
    T = np.zeros((NW, NROUND), np.int64)
    for k in range(NROUND):
        best = None
        for t1 in range(int(a1[k]), int(aall[k]) + 1):
            for t3 in range(int(a5[k]), int(aall[k]) + 1):
                if t1 + t3 < a15[k]:
                    continue
                t2 = max(0, int(a12[k]) - t1, int(a45[k]) - t3,
                         int(aall[k]) - t1 - t3)
                s = t1 + t2 + t3
                if best is None or s < best[0]:
                    best = (s, t1, t2, t3)
            if best is not None and best[0] == aall[k]:
                break
        T[0, k], T[1, k], T[2, k] = best[1], best[2], best[3]

    # per-slot greedy assignment (fill side windows first)
    slot_round = np.arange(8 * SHARD) % SHARD // 128
    T1s = T[0][slot_round]
    T2s = T[1][slot_round]
    T3s = T[2][slot_round]
    n1, n2, n3, n4, n5 = cnt_r
    r2A = np.minimum(np.maximum(T1s - n1, 0), n2)
    r4C = np.minimum(np.maximum(T3s - n5, 0), n4)
    r3A = np.minimum(np.maximum(T1s - n1 - r2A, 0), n3)
    r3C = np.minimum(np.maximum(T3s - n5 - r4C, 0), n3 - r3A)
    useA = n1 + r2A + r3A
    useC = n5 + r4C + r3C
    useB = tot - useA - useC
    assert (useA <= T1s).all() and (useC <= T3s).all() and (useB <= T2s).all()

    # trim T to the actual per-round maxima after assignment
    T = np.stack([rmax(useA), rmax(useB), rmax(useC)])
    callbase = [np.concatenate([[0], np.cumsum(T[w])]) for w in range(NW)]
    CALLS = [int(callbase[w][-1]) for w in range(NW)]

    # per-edge window assignment via per-(slot, region) quotas
    order = np.lexsort((region, key))
    k_sorted = key[order]
    reg_sorted = region[order]
    grp = k_sorted * 5 + reg_sorted
    seq = (np.arange(len(grp))
           - np.concatenate([[0], np.cumsum(np.bincount(grp))])[grp])
    win = np.empty(len(order), np.int64)
    r = reg_sorted
    win[r == 0] = 0
    win[r == 4] = 2
    win[r == 1] = np.where(seq[r == 1] < r2A[k_sorted[r == 1]], 0, 1)
    win[r == 3] = np.where(seq[r == 3] < r4C[k_sorted[r == 3]], 2, 1)
    s3 = seq[r == 2]
    k3 = k_sorted[r == 2]
    win[r == 2] = np.where(s3 < r3A[k3], 0,
                           np.where(s3 < (r3A + r3C)[k3], 2, 1))

    # padded per-core per-window call tables -> wrapped int16 index arrays
    rows_s = rows[order]
    dslot_s = d_slot[order]
    rank_s = e_rank[order]
    idx16s = [[None] * NW for _ in range(NCORES)]
    for c in range(NCORES):
        for w in range(NW):
            m = (rank_s == c) & (win == w)
            rw = rows_s[m] - WBASE[w]
            ds = dslot_s[m]
            o2 = np.argsort(ds, kind="stable")
            rw = rw[o2]
            ds = ds[o2]
            cw = np.bincount(ds, minlength=SHARD)
            off = np.concatenate([[0], np.cumsum(cw)])
            tab = np.full((128, CALLS[w]), GAPROW - WBASE[w], np.int64)
            j = np.arange(len(ds)) - off[ds]
            kk = ds // 128
            p = ds % 128
            tab[p, callbase[w][kk] + j] = rw
            L = tab.T.reshape(-1)  # linear order: call-major, partition-minor
            blk = L.reshape(-1, 16).T.astype(np.int16)  # [16, 8*CALLS]
            idx16s[c][w] = np.ascontiguousarray(np.tile(blk, (8, 1)))

    u0_full = np.zeros((NTOT2, F), np.float32)
    u0_full[pidx] = x * dinv[:, None]
    dinv_nms = []
    for c in range(NCORES):
        nodes = np.arange(NPG * GSTART[c], NPG * GSTART[c + 1])
        dv = np.zeros(SHARD, np.float32)
        dv[slot_of[nodes]] = dinv[nodes]
        dinv_nms.append(np.ascontiguousarray(dv.reshape(NROUND, 128).T))

    return T, callbase, CALLS, idx16s, u0_full, dinv_nms


def _chunks(T, callbase):
    """Round-aligned chunks with at most CH gather columns per window."""
    out = []
    k0 = 0
    while k0 < NROUND:
        k1 = k0
        while k1 < NROUND and all(
                callbase[w][k1 + 1] - callbase[w][k0] <= CH for w in range(NW)):
            k1 += 1
        if k1 == k0:
            k1 = k0 + 1
        out.append((k0, k1))
        k0 = k1
    return out


def _pack_classifier(inp):
    Wc0 = np.asarray(inp["Wc0"], np.float32)           # [2895, 256]
    Wc0r = Wc0.reshape(K_TOP, CAT, 256)
    WA = np.ascontiguousarray(Wc0r[:, 0:128, :].transpose(1, 0, 2))
    WB = np.zeros((80, K_TOP, 256), np.float32)
    WB[0:65] = Wc0r[:, 128:193, :].transpose(1, 0, 2)
    sc_full = np.asarray(inp["gamma"], np.float32) * np.float32(
        1.0 / np.sqrt(1.0 + BN_EPS))
    be_full = (np.asarray(inp["beta"], np.float32)
               + np.asarray(inp["bc0"], np.float32) * sc_full)
    sc = np.ascontiguousarray(sc_full.reshape(2, 128).T)
    be = np.ascontiguousarray(be_full.reshape(2, 128).T)
    Wc1 = np.asarray(inp["Wc1"], np.float32)
    Wc1s = np.ascontiguousarray(Wc1.reshape(2, 128, 128).transpose(1, 0, 2))
    return {
        "WA": WA, "WB": WB, "sc": sc, "be": be, "Wc1s": Wc1s,
        "bc1": np.asarray(inp["bc1"], np.float32).reshape(128, 1),
        "Wc2": np.asarray(inp["Wc2"], np.float32),
        "bc2": np.asarray(inp["bc2"], np.float32).reshape(64, 1),
        "Wc3": np.asarray(inp["Wc3"], np.float32),
        "bc3": np.asarray(inp["bc3"], np.float32).reshape(1, 1),
    }


def _build(T, callbase, CALLS):
    import concourse.bass as bass
    import concourse.bacc as bacc
    import concourse.mybir as mybir
    from concourse import tile
    from concourse.masks import make_identity

    f32 = mybir.dt.float32
    i16 = mybir.dt.int16
    AF = mybir.ActivationFunctionType
    chunks = _chunks(T, callbase)

    nc = bacc.Bacc("TRN2", target_bir_lowering=False, debug=False,
                   num_devices=NCORES)

    # ---- I/O ----
    idx_d = [nc.dram_tensor(f"idx16_{w}", [128, 8 * CALLS[w]], i16,
                            kind="ExternalInput") for w in range(NW)]
    dinv_d = nc.dram_tensor("dinv_nm", [128, NROUND], f32, kind="ExternalInput")
    W_d = [nc.dram_tensor(f"W{i}", [F, F if i < 3 else 1], f32,
                          kind="ExternalInput") for i in range(4)]
    b_d = [nc.dram_tensor(f"b{i}", [F if i < 3 else 1, 1], f32,
                          kind="ExternalInput") for i in range(4)]
    WA_d = nc.dram_tensor("WA", [128, K_TOP, 256], f32, kind="ExternalInput")
    WB_d = nc.dram_tensor("WB", [80, K_TOP, 256], f32, kind="ExternalInput")
    sc_d = nc.dram_tensor("sc", [128, 2], f32, kind="ExternalInput")
    be_d = nc.dram_tensor("be", [128, 2], f32, kind="ExternalInput")
    Wc1_d = nc.dram_tensor("Wc1s", [128, 2, 128], f32, kind="ExternalInput")
    bc1_d = nc.dram_tensor("bc1", [128, 1], f32, kind="ExternalInput")
    Wc2_d = nc.dram_tensor("Wc2", [128, F], f32, kind="ExternalInput")
    bc2_d = nc.dram_tensor("bc2", [F, 1], f32, kind="ExternalInput")
    Wc3_d = nc.dram_tensor("Wc3", [F, 1], f32, kind="ExternalInput")
    bc3_d = nc.dram_tensor("bc3", [1, 1], f32, kind="ExternalInput")
    out_d = nc.dram_tensor("out", [1, 13], f32, kind="ExternalOutput")

    u0full_d = nc.dram_tensor("u0_full", [NTOT2, F], f32,
                              kind="ExternalInput")
    bounceA = nc.dram_tensor("bounceA", [3328, F], f32, kind="Internal")
    bounceB = nc.dram_tensor("bounceB", [1664, F], f32, kind="Internal")
    bounceC = nc.dram_tensor("bounceC", [1664, F], f32, kind="Internal")
    u_alls = [nc.dram_tensor(f"u_all{i}", [NTOT2, F], f32, kind="Internal",
                             addr_space="Shared") for i in range(2)]
    d6656 = nc.dram_tensor("d6656", [1, SHARD], f32, kind="Internal")
    d208 = nc.dram_tensor("d208", [1, 208], mybir.dt.int16, kind="Internal")

    with tile.TileContext(nc) as tc:
        with (
            tc.tile_pool(name="persist", bufs=1) as pp,
            tc.tile_pool(name="psum_t", bufs=4, space="PSUM") as ps_t,
            tc.tile_pool(name="psum_w", bufs=2, space="PSUM") as ps_w,
        ):
            ident = pp.tile([128, 128], f32)
            make_identity(nc, ident[:])
            idx16 = []
            for w in range(NW):
                t = pp.tile([128, 8 * CALLS[w]], i16, name=f"idx16_{w}")
                nc.sync.dma_start(t[:], idx_d[w][:])
                idx16.append(t)
            dinv = pp.tile([128, NROUND], f32)
            nc.sync.dma_start(dinv[:], dinv_d[:])
            Ws, bs = [], []
            for i in range(4):
                w = pp.tile([F, F if i < 3 else 1], f32, name=f"W{i}s")
                nc.sync.dma_start(w[:], W_d[i][:])
                Ws.append(w)
                bb = pp.tile([F if i < 3 else 1, 1], f32, name=f"b{i}s")
                nc.sync.dma_start(bb[:], b_d[i][:])
                bs.append(bb)
            hcatA = pp.tile([128, SHARD], f32)   # h0 (rows 0:64), h1 (64:128)
            hcatB = pp.tile([80, SHARD], f32)    # h2 (0:64), h3 (row 64)

            # zero the gap rows of both internal tables
            zt = pp.tile([128, F], f32)
            nc.vector.memset(zt[:], 0.0)
            for i in range(2):
                nc.sync.dma_start(u_alls[i].ap()[GAPROW:GAPROW + 128, :], zt[:])

            with tc.tile_pool(name="conv", bufs=1) as cp:
                def coll_piece(dst, which):
                    src = (bounceA, bounceB, bounceC)[which]
                    base = PBASE[which]
                    rows = (3328, 1664, 1664)[which]
                    nc.gpsimd.collective_compute(
                        "AllGather", mybir.AluOpType.bypass,
                        replica_groups=[list(range(NCORES))],
                        ins=[src.ap()],
                        outs=[dst.ap()[base:base + NCORES * rows, :]])

                # layer-0 table comes prebuilt from the host; collectives for
                # layer l+1 write the alternate table while layer l gathers.
                tabs = [u0full_d, u_alls[1], u_alls[0], u_alls[1]]
                u_next = None
                for layer in range(4):
                    u_all = tabs[layer]

                    fo = F if layer < 3 else 1
                    hout = (hcatA[0:64, :] if layer == 0 else
                            hcatA[64:128, :] if layer == 1 else
                            hcatB[0:64, :] if layer == 2 else
                            hcatB[64:65, :])
                    if layer < 3:
                        u_next = cp.tile([128, NROUND, F], f32, tag="u_nm",
                                         name=f"u_nm{layer + 1}", bufs=2)
                    s_fm = None

                    collA_issued = 0
                    for (k0, k1) in chunks:
                        gts = [None] * NW
                        for w in range(NW):
                            c0 = int(callbase[w][k0])
                            c1 = int(callbase[w][k1])
                            if c1 == c0:
                                continue
                            gt = cp.tile([128, CH, F], f32, tag=f"gt{w}",
                                         name=f"gt{w}_{layer}_{k0}", bufs=2)
                            gts[w] = (gt, c0)
                            nc.gpsimd.dma_gather(
                                gt[:, 0:c1 - c0, :],
                                u_all.ap()[WBASE[w]:WBASE[w] + WIN, :],
                                idx16[w][:, 8 * c0:8 * c1],
                                num_idxs=128 * (c1 - c0),
                                num_idxs_reg=128 * (c1 - c0),
                                elem_size=F,
                                single_packet=False)
                        for k in range(k0, k1):
                            j = k % 4
                            if j == 0 or s_fm is None:
                                s_fm = cp.tile([F, 512], f32, tag="s_fm",
                                               name=f"s_fm{layer}_{k}", bufs=2)
                            roots = []
                            for w in range(NW):
                                dk = int(T[w][k])
                                if dk == 0:
                                    continue
                                q0 = int(callbase[w][k]) - gts[w][1]
                                gt = gts[w][0]
                                if dk == 1:
                                    roots.append(gt[:, q0, :])
                                    continue
                                # level 1 into scratch (gather tile stays
                                # read-only), then in-place tree on scratch
                                b = dk // 2
                                a = dk - b
                                rr = cp.tile([128, (CH + 1) // 2, F], f32,
                                             tag=f"red{w}",
                                             name=f"rd{w}_{layer}_{k}", bufs=2)
                                nc.vector.tensor_tensor(
                                    out=rr[:, 0:b, :],
                                    in0=gt[:, q0:q0 + b, :],
                                    in1=gt[:, q0 + a:q0 + dk, :],
                                    op=mybir.AluOpType.add)
                                if a > b:
                                    roots.append(gt[:, q0 + b, :])
                                d_cur = b
                                while d_cur > 1:
                                    h = d_cur // 2
                                    nc.vector.tensor_tensor(
                                        out=rr[:, 0:h, :],
                                        in0=rr[:, 0:h, :],
                                        in1=rr[:, d_cur - h:d_cur, :],
                                        op=mybir.AluOpType.add)
                                    d_cur -= h
                                roots.append(rr[:, 0, :])
                            if not roots:
                                nc.vector.memset(
                                    s_fm[:, j * 128:(j + 1) * 128], 0.0)
                            else:
                                s_nm = cp.tile([128, F], f32, tag="s_nm",
                                               name=f"s_nm{layer}_{k}", bufs=4)
                                acc = roots[0]
                                for rt in roots[1:]:
                                    nc.vector.tensor_tensor(
                                        out=s_nm[:], in0=acc, in1=rt,
                                        op=mybir.AluOpType.add)
                                    acc = s_nm[:]
                                nc.vector.tensor_scalar(
                                    out=s_nm[:], in0=acc,
                                    scalar1=dinv[:, k:k + 1], scalar2=None,
                                    op0=mybir.AluOpType.mult)
                                ptile = ps_t.tile([F, 128], f32, tag="trf",
                                                  name=f"ptf{layer}_{k}",
                                                  bufs=2)
                                nc.tensor.matmul(ptile[:], s_nm[:], ident[:],
                                                 is_transpose=True)
                                nc.scalar.activation(
                                    s_fm[:, j * 128:(j + 1) * 128], ptile[:],
                                    AF.Copy)
                            # transform + tanh + inline staging per 4 rounds
                            if j == 3 or k == NROUND - 1:
                                t0 = (k // 4) * 4
                                nc0 = t0 * 128
                                nc1 = (k + 1) * 128
                                pw = ps_w.tile([fo, 512], f32, tag="pw",
                                               name=f"pw{layer}_{t0}")
                                nc.tensor.matmul(pw[:, 0:nc1 - nc0],
                                                 Ws[layer][:],
                                                 s_fm[:, 0:nc1 - nc0],
                                                 start=True, stop=True)
                                nc.scalar.activation(
                                    hout[:, nc0:nc1], pw[:, 0:nc1 - nc0],
                                    AF.Tanh, bias=bs[layer][:])
                                if layer < 3:
                                    for t in range(t0, k + 1):
                                        pt2 = ps_t.tile(
                                            [128, F], f32, tag="trs",
                                            name=f"pts{layer}_{t}", bufs=2)
                                        ident64 = (ident[64:128, 64:128]
                                                   if layer == 1
                                                   else ident[0:F, 0:F])
                                        nc.tensor.matmul(
                                            pt2[:],
                                            hout[:, t * 128:(t + 1) * 128],
                                            ident64,
                                            is_transpose=True)
                                        nc.vector.tensor_tensor(
                                            out=u_next[:, t, :], in0=pt2[:],
                                            in1=dinv[:, t:t + 1]
                                            .to_broadcast([128, F]),
                                            op=mybir.AluOpType.mult)
                        if layer < 3 and collA_issued == 0 and k1 >= 28:
                            # rounds 0-27 staged: ship piece A of next table
                            # while the rest of this layer still gathers.
                            nc.sync.dma_start(
                                bounceA.ap().rearrange(
                                    "(t p) f -> p t f", p=128),
                                u_next[:, 0:26, :])
                            coll_piece(u_alls[(layer + 1) % 2], 0)
                            collA_issued = 1
                        if layer < 3 and collA_issued == 1 and k1 >= 40:
                            nc.sync.dma_start(
                                bounceB.ap().rearrange(
                                    "(t p) f -> p t f", p=128),
                                u_next[:, 26:39, :])
                            coll_piece(u_alls[(layer + 1) % 2], 1)
                            collA_issued = 2
                    if layer < 3:
                        nc.sync.dma_start(
                            bounceC.ap().rearrange("(t p) f -> p t f", p=128),
                            u_next[:, 39:NROUND, :])
                        coll_piece(u_alls[(layer + 1) % 2], 2)

            # ================= sort-pool + classifier =================
            with tc.tile_pool(name="poolp", bufs=1) as qp:
                WA = qp.tile([128, K_TOP, 256], f32)
                nc.sync.dma_start(WA[:], WA_d[:])
                WB = qp.tile([80, K_TOP, 256], f32)
                nc.sync.dma_start(WB[:], WB_d[:])
                sc = qp.tile([128, 2], f32)
                nc.sync.dma_start(sc[:], sc_d[:])
                be = qp.tile([128, 2], f32)
                nc.sync.dma_start(be[:], be_d[:])
                Wc1s = qp.tile([128, 2, 128], f32)
                nc.sync.dma_start(Wc1s[:], Wc1_d[:])
                bc1 = qp.tile([128, 1], f32)
                nc.sync.dma_start(bc1[:], bc1_d[:])
                Wc2 = qp.tile([128, F], f32)
                nc.sync.dma_start(Wc2[:], Wc2_d[:])
                bc2 = qp.tile([F, 1], f32)
                nc.sync.dma_start(bc2[:], bc2_d[:])
                Wc3 = qp.tile([F, 1], f32)
                nc.sync.dma_start(Wc3[:], Wc3_d[:])
                bc3 = qp.tile([1, 1], f32)
                nc.sync.dma_start(bc3[:], bc3_d[:])

                h3 = hcatB[64:65, :]
                nc.sync.dma_start(d6656.ap(), h3)
                h3gt = qp.tile([13, NPG], f32)
                nc.sync.dma_start(
                    h3gt[:],
                    d6656.ap()[:, 0:6500].rearrange(
                        "one (i g) -> (one g) i", g=13))
                h3g = h3gt[:]

                m8a = qp.tile([13, 8], f32)
                i8a = qp.tile([13, 8], mybir.dt.uint32)
                nc.vector.max(m8a[:], h3g)
                nc.vector.max_index(i8a[:], m8a[:], h3g)
                h3m = qp.tile([13, NPG], f32)
                nc.vector.match_replace(h3m[:], m8a[:], h3g, -2.0)
                m8b = qp.tile([13, 8], f32)
                i8b = qp.tile([13, 8], mybir.dt.uint32)
                nc.vector.max(m8b[:], h3m[:])
                nc.vector.max_index(i8b[:], m8b[:], h3m[:])

                idx2d = qp.tile([13, 16], f32)
                nc.vector.memset(idx2d[:], 0.0)
                nc.vector.tensor_copy(idx2d[:, 0:8], i8a[:])
                nc.vector.tensor_copy(idx2d[:, 8:15], i8b[:, 0:7])
                gof = qp.tile([13, 1], mybir.dt.int32)
                nc.gpsimd.iota(gof[:], [[0, 1]], base=0, channel_multiplier=1)
                goff = qp.tile([13, 1], f32)
                nc.vector.tensor_copy(goff[:], gof[:])
                # absolute slot = pos*13 + g
                nc.vector.tensor_scalar(
                    out=idx2d[:], in0=idx2d[:], scalar1=13.0,
                    scalar2=None, op0=mybir.AluOpType.mult)
                nc.vector.tensor_scalar(
                    out=idx2d[:], in0=idx2d[:], scalar1=goff[:, 0:1],
                    scalar2=None, op0=mybir.AluOpType.add)
                idx16p = qp.tile([13, 16], mybir.dt.int16)
                nc.vector.tensor_copy(idx16p[:], idx2d[:])
                nc.sync.dma_start(
                    d208.ap().rearrange("one (g r) -> (one g) r", g=13),
                    idx16p[:])
                idx128 = qp.tile([128, 13], mybir.dt.int16)
                for kk in range(8):
                    nc.sync.dma_start(
                        idx128[kk * 16:(kk + 1) * 16, :],
                        d208.ap().rearrange("one (s p) -> (one p) s", p=16))

                poolA = qp.tile([128, 208], f32)
                nc.gpsimd.ap_gather(poolA[:], hcatA[:], idx128[:],
                                    channels=128, num_elems=SHARD, d=1,
                                    num_idxs=208)
                poolB = qp.tile([80, 208], f32)
                nc.gpsimd.ap_gather(poolB[:], hcatB[:], idx128[0:80, :],
                                    channels=80, num_elems=SHARD, d=1,
                                    num_idxs=208)

                z1 = []
                for mh in range(2):
                    pz = ps_w.tile([128, 13], f32, tag="pw", name=f"pz{mh}")
                    first = True
                    for r in range(K_TOP):
                        nc.tensor.matmul(
                            pz[:], WA[:, r, mh * 128:(mh + 1) * 128],
                            poolA[:, r:r + 16 * 12 + 1:16],
                            start=first, stop=False)
                        first = False
                        nc.tensor.matmul(
                            pz[:], WB[0:65, r, mh * 128:(mh + 1) * 128],
                            poolB[0:65, r:r + 16 * 12 + 1:16],
                            start=False, stop=(r == K_TOP - 1))
                    zz = qp.tile([128, 13], f32, tag=f"z1_{mh}", name=f"z1_{mh}")
                    nc.scalar.activation(zz[:], pz[:], AF.Relu,
                                         bias=be[:, mh:mh + 1],
                                         scale=sc[:, mh:mh + 1])
                    z1.append(zz)
                pz2 = ps_w.tile([128, 13], f32, tag="pw", name="pz2")
                nc.tensor.matmul(pz2[:], Wc1s[:, 0, :], z1[0][:],
                                 start=True, stop=False)
                nc.tensor.matmul(pz2[:], Wc1s[:, 1, :], z1[1][:],
                                 start=False, stop=True)
                z2 = qp.tile([128, 13], f32)
                nc.scalar.activation(z2[:], pz2[:], AF.Relu, bias=bc1[:])
                pz3 = ps_w.tile([F, 13], f32, tag="pw", name="pz3")
                nc.tensor.matmul(pz3[:], Wc2[:], z2[:], start=True, stop=True)
                z3 = qp.tile([F, 13], f32)
                nc.scalar.activation(z3[:], pz3[:], AF.Relu, bias=bc2[:])
                pz4 = ps_w.tile([1, 13], f32, tag="pw", name="pz4")
                nc.tensor.matmul(pz4[:], Wc3[:], z3[:], start=True, stop=True)
                zf = qp.tile([1, 13], f32)
                nc.vector.tensor_scalar(out=zf[:], in0=pz4[:],
                                        scalar1=bc3[0:1, 0:1], scalar2=None,
                                        op0=mybir.AluOpType.add)
                nc.sync.dma_start(out_d[:], zf[:])

    nc.compile()
    return nc


def kernel(**inputs):
    from concourse import bass_utils

    x = np.asarray(inputs["x"], np.float32)
    edge_index = np.asarray(inputs["edge_index"])

    T, callbase, CALLS, idx16s, u0_full, dinv_nms = _prep(x, edge_index)
    key = ("prog",)
    tkey = T.tobytes()
    if key in _CACHE and _CACHE[key][0] == tkey:
        nc = _CACHE[key][1]
    else:
        nc = _build(T, callbase, CALLS)
        _CACHE[key] = (tkey, nc)

    cw = _pack_classifier(inputs)
    in_maps = []
    for c in range(NCORES):
        m = {
            "u0_full": u0_full,
            "dinv_nm": dinv_nms[c],
            "WA": cw["WA"], "WB": cw["WB"], "sc": cw["sc"], "be": cw["be"],
            "Wc1s": cw["Wc1s"], "bc1": cw["bc1"], "Wc2": cw["Wc2"],
            "bc2": cw["bc2"], "Wc3": cw["Wc3"], "bc3": cw["bc3"],
        }
        for w in range(NW):
            m[f"idx16_{w}"] = idx16s[c][w]
        for i in range(4):
            m[f"W{i}"] = np.asarray(inputs[f"W{i}"], np.float32).reshape(
                F, F if i < 3 else 1)
            m[f"b{i}"] = np.asarray(inputs[f"b{i}"], np.float32).reshape(
                F if i < 3 else 1, 1)
        in_maps.append(m)

    trace = os.environ.get("KERNEL_TRACE", "0") == "1"
    kwargs = {}
    if trace:
        import sys, types
        if "antenv.axon_hooks" not in sys.modules:
            sys.path.insert(0, "/root/.axon_site")
            from trn_agent_boot.trn_boot import _ntff_profile_via_ctypes
            mm = types.ModuleType("antenv.axon_hooks")
            mm.get_axon_ntff_profile_hook = (
                lambda: _ntff_profile_via_ctypes("/opt/axon/libaxon_pjrt.so"))
            sys.modules["antenv.axon_hooks"] = mm
        import tempfile
        kwargs = dict(trace=True, tmpdir=tempfile.mkdtemp())

    res = bass_utils.run_bass_kernel_spmd(
        nc, in_maps, core_ids=list(range(NCORES)), **kwargs)

    global LAST_EXEC_NS
    LAST_EXEC_NS = res.exec_time_ns

    out = np.zeros((G, 1), np.float32)
    for c in range(NCORES):
        ngr = GRAPHS_PER_CORE[c]
        out[GSTART[c]:GSTART[c] + ngr, 0] = res.results[c]["out"][0, :ngr]
    return out


LAST_EXEC_NS = None


# revision 7
# speedup vs baseline: 1.1518x; 1.1518x over previous
"""DGCNN (4x GCNConv + sort-pool + MLP) on 8 trn2 NeuronCores.

Strategy: graph-parallel sharding (ranks 0-3: 13 graphs, 4-7: 12).
Interleaved slot layout (slot = per-graph degree rank * 13 + graph) so
each 128-dst round holds a narrow degree band across all graphs (fewer
padded slab columns than per-graph blocks) while the sort-pool reload
stays a pure affine DMA. Per layer: u = dinv*h staged node-major (u0
prebuilt on host) -> AllGather full f32 table -> per-slab-column
indirect row gathers into chunked SBUF tiles -> in-place vector-engine
tree reduction per round (+self-loop, dinv scale) -> PE transpose ->
feature transform + tanh per 4 rounds with inline staging of the next
layer's table. Sort-pool via max8/max_index/match_replace, pooled rows
extracted with ap_gather, classifier on PE.
"""
import os
import numpy as np

N = 50000
G = 100
NPG = 500
E = 800000
F = 64
K_TOP = 15
CAT = 193
NCORES = 8
SHARD = 6656
NTOT = SHARD * NCORES
NROUND = SHARD // 128  # 52
BN_EPS = 1e-5
ZROW = 41599           # core-0 pad slot 6655 in 3-piece table rows
CH_MAX = 112           # max gather columns per chunk

GRAPHS_PER_CORE = [13, 13, 13, 13, 12, 12, 12, 12]
GSTART = np.concatenate([[0], np.cumsum(GRAPHS_PER_CORE)])

_CACHE = {}


def _prep(x, edge_index):
    """Host-side sharding/index preprocessing. Pure numpy."""
    src = edge_index[0].astype(np.int64)
    dst = edge_index[1].astype(np.int64)

    deg = np.bincount(dst, minlength=N).astype(np.float32) + np.float32(1.0)
    dinv = deg ** np.float32(-0.5)
    indeg = np.bincount(dst, minlength=N).astype(np.int64)

    node_graph = np.arange(N) // NPG
    node_rank = np.searchsorted(GSTART, node_graph, side="right") - 1  # [N]

    # interleaved slot layout: per-graph degree rank (desc) r, local graph
    # g -> slot = r*13 + g. Rounds then hold narrow degree-quantile bands
    # across all graphs, and the pooling un-permute is a pure affine reload.
    slot_of = np.zeros(N, np.int64)
    for g in range(G):
        lo = g * NPG
        o = np.argsort(-indeg[lo:lo + NPG], kind="stable")
        r = np.empty(NPG, np.int64)
        r[o] = np.arange(NPG)
        g_local = g - GSTART[np.searchsorted(GSTART, g, side="right") - 1]
        slot_of[lo:lo + NPG] = r * 13 + g_local

    # 3-piece table rows (A: slots 0:3328, B: 3328:4992, C: 4992:6656),
    # each piece AllGather'd separately so staging can ship early pieces
    # while later rounds still gather. Row = base[p] + rank*rows[p] +
    # (slot - lo[p]).
    piece = ((slot_of >= 3328).astype(np.int64)
             + (slot_of >= 4992).astype(np.int64))
    pbase = np.array([0, 26624, 39936], np.int64)
    prows = np.array([3328, 1664, 1664], np.int64)
    plo = np.array([0, 3328, 4992], np.int64)
    pidx = pbase[piece] + node_rank * prows[piece] + (slot_of - plo[piece])

    # per-core edge lists grouped by dst slot
    e_rank = node_rank[dst]
    counts_all = np.zeros((NCORES, SHARD), np.int64)
    per_core = []
    for c in range(NCORES):
        m = e_rank == c
        s_p = pidx[src[m]].astype(np.int64)
        d_slot = slot_of[dst[m]]
        o = np.argsort(d_slot, kind="stable")
        s_p = s_p[o]
        d_slot = d_slot[o]
        cnt = np.bincount(d_slot, minlength=SHARD)
        counts_all[c] = cnt
        per_core.append((s_p, d_slot, cnt))

    D = counts_all.reshape(NCORES, NROUND, 128).max(axis=(0, 2))  # [NROUND]
    callbase = np.concatenate([[0], np.cumsum(D)])
    CALLS = int(callbase[-1])

    u0_full = np.zeros((NCORES * SHARD, F), np.float32)
    gidxs, u0bs, dinv_nms = [], [], []
    for c in range(NCORES):
        s_p, d_slot, cnt = per_core[c]
        off = np.concatenate([[0], np.cumsum(cnt)])
        gid = np.full((128, CALLS), ZROW, np.int32)
        j_within = np.arange(len(d_slot)) - off[d_slot]
        k = d_slot // 128
        p = d_slot % 128
        call = callbase[k] + j_within
        gid[p, call] = s_p.astype(np.int32)
        gidxs.append(gid)

        nodes = np.arange(NPG * GSTART[c], NPG * GSTART[c + 1])
        u0 = np.zeros((SHARD, F), np.float32)
        u0[slot_of[nodes]] = x[nodes] * dinv[nodes][:, None]
        u0bs.append(u0)
        for p in range(3):
            lo, rows = int(plo[p]), int(prows[p])
            u0_full[int(pbase[p]) + c * rows:
                    int(pbase[p]) + (c + 1) * rows] = u0[lo:lo + rows]

        dv = np.zeros(SHARD, np.float32)
        dv[slot_of[nodes]] = dinv[nodes]
        dinv_nms.append(np.ascontiguousarray(dv.reshape(NROUND, 128).T))

    return CALLS, D, callbase, gidxs, u0bs, dinv_nms, u0_full


def _chunks(D, callbase):
    """Round-aligned column chunks of at most CH_MAX columns."""
    out = []
    k0 = 0
    while k0 < NROUND:
        k1 = k0
        cols = 0
        while k1 < NROUND and cols + int(D[k1]) <= CH_MAX:
            cols += int(D[k1])
            k1 += 1
        if k1 == k0:  # single round exceeding CH_MAX (cannot happen: D<=128)
            k1 = k0 + 1
            cols = int(D[k0])
        out.append((k0, k1, int(callbase[k0]), int(callbase[k1])))
        k0 = k1
    k0, k1, c0, c1 = out[-1]
    if k1 - k0 > 3:  # short final chunk -> shorter post-gather tail
        cm = int(callbase[k1 - 2])
        out[-1] = (k0, k1 - 2, c0, cm)
        out.append((k1 - 2, k1, cm, c1))
    return out


def _pack_classifier(inp):
    Wc0 = np.asarray(inp["Wc0"], np.float32)           # [2895, 256]
    Wc0r = Wc0.reshape(K_TOP, CAT, 256)
    WA = np.ascontiguousarray(Wc0r[:, 0:128, :].transpose(1, 0, 2))   # [128,15,256]
    WB = np.zeros((80, K_TOP, 256), np.float32)
    WB[0:65] = Wc0r[:, 128:193, :].transpose(1, 0, 2)
    sc_full = np.asarray(inp["gamma"], np.float32) * np.float32(
        1.0 / np.sqrt(1.0 + BN_EPS))
    be_full = (np.asarray(inp["beta"], np.float32)
               + np.asarray(inp["bc0"], np.float32) * sc_full)
    sc = np.ascontiguousarray(sc_full.reshape(2, 128).T)  # [128, 2]
    be = np.ascontiguousarray(be_full.reshape(2, 128).T)
    Wc1 = np.asarray(inp["Wc1"], np.float32)              # [256, 128]
    Wc1s = np.ascontiguousarray(Wc1.reshape(2, 128, 128).transpose(1, 0, 2))
    return {
        "WA": WA, "WB": WB, "sc": sc, "be": be, "Wc1s": Wc1s,
        "bc1": np.asarray(inp["bc1"], np.float32).reshape(128, 1),
        "Wc2": np.asarray(inp["Wc2"], np.float32),
        "bc2": np.asarray(inp["bc2"], np.float32).reshape(64, 1),
        "Wc3": np.asarray(inp["Wc3"], np.float32),
        "bc3": np.asarray(inp["bc3"], np.float32).reshape(1, 1),
    }


def _build(CALLS, D, callbase):
    import concourse.bass as bass
    import concourse.bacc as bacc
    import concourse.mybir as mybir
    from concourse import tile
    from concourse.masks import make_identity

    f32 = mybir.dt.float32
    i32 = mybir.dt.int32
    AF = mybir.ActivationFunctionType
    chunks = _chunks(D, callbase)

    nc = bacc.Bacc("TRN2", target_bir_lowering=False, debug=False,
                   num_devices=NCORES)

    # ---- I/O ----
    u0b_d = nc.dram_tensor("u0b", [SHARD, F], f32, kind="ExternalInput")
    gidx_d = nc.dram_tensor("gidx", [128, CALLS], i32, kind="ExternalInput")
    dinv_d = nc.dram_tensor("dinv_nm", [128, NROUND], f32, kind="ExternalInput")
    W_d = [nc.dram_tensor(f"W{i}", [F, F if i < 3 else 1], f32,
                          kind="ExternalInput") for i in range(4)]
    b_d = [nc.dram_tensor(f"b{i}", [F if i < 3 else 1, 1], f32,
                          kind="ExternalInput") for i in range(4)]
    WA_d = nc.dram_tensor("WA", [128, K_TOP, 256], f32, kind="ExternalInput")
    WB_d = nc.dram_tensor("WB", [80, K_TOP, 256], f32, kind="ExternalInput")
    sc_d = nc.dram_tensor("sc", [128, 2], f32, kind="ExternalInput")
    be_d = nc.dram_tensor("be", [128, 2], f32, kind="ExternalInput")
    Wc1_d = nc.dram_tensor("Wc1s", [128, 2, 128], f32, kind="ExternalInput")
    bc1_d = nc.dram_tensor("bc1", [128, 1], f32, kind="ExternalInput")
    Wc2_d = nc.dram_tensor("Wc2", [128, F], f32, kind="ExternalInput")
    bc2_d = nc.dram_tensor("bc2", [F, 1], f32, kind="ExternalInput")
    Wc3_d = nc.dram_tensor("Wc3", [F, 1], f32, kind="ExternalInput")
    bc3_d = nc.dram_tensor("bc3", [1, 1], f32, kind="ExternalInput")
    out_d = nc.dram_tensor("out", [1, 13], f32, kind="ExternalOutput")

    u0full_d = nc.dram_tensor("u0_full", [NTOT, F], f32,
                              kind="ExternalInput")
    bounceA = nc.dram_tensor("bounceA", [3328, F], f32, kind="Internal")
    bounceB = nc.dram_tensor("bounceB", [1664, F], f32, kind="Internal")
    bounceC = nc.dram_tensor("bounceC", [1664, F], f32, kind="Internal")
    u_alls = [nc.dram_tensor(f"u_all{i}", [NTOT, F], f32, kind="Internal",
                             addr_space="Shared") for i in range(2)]
    d6656 = nc.dram_tensor("d6656", [1, SHARD], f32, kind="Internal")
    d208 = nc.dram_tensor("d208", [1, 208], mybir.dt.int16, kind="Internal")

    with tile.TileContext(nc) as tc:
        with (
            tc.tile_pool(name="persist", bufs=1) as pp,
            tc.tile_pool(name="psum_t", bufs=4, space="PSUM") as ps_t,
            tc.tile_pool(name="psum_w", bufs=2, space="PSUM") as ps_w,
        ):
            ident = pp.tile([128, 128], f32)
            make_identity(nc, ident[:])
            gidx = pp.tile([128, CALLS], i32)
            nc.sync.dma_start(gidx[:], gidx_d[:])
            dinv = pp.tile([128, NROUND], f32)
            nc.sync.dma_start(dinv[:], dinv_d[:])
            Ws, bs = [], []
            for i in range(4):
                w = pp.tile([F, F if i < 3 else 1], f32, name=f"W{i}s")
                nc.sync.dma_start(w[:], W_d[i][:])
                Ws.append(w)
                bb = pp.tile([F if i < 3 else 1, 1], f32, name=f"b{i}s")
                nc.sync.dma_start(bb[:], b_d[i][:])
                bs.append(bb)
            hcatA = pp.tile([128, SHARD], f32)   # h0 (rows 0:64), h1 (64:128)
            hcatB = pp.tile([80, SHARD], f32)    # h2 (0:64), h3 (row 64)
            WA = pp.tile([128, K_TOP, 256], f32)
            nc.sync.dma_start(WA[:], WA_d[:])
            WB = pp.tile([80, K_TOP, 256], f32)
            nc.sync.dma_start(WB[:], WB_d[:])
            sc = pp.tile([128, 2], f32)
            nc.sync.dma_start(sc[:], sc_d[:])
            be = pp.tile([128, 2], f32)
            nc.sync.dma_start(be[:], be_d[:])
            Wc1s = pp.tile([128, 2, 128], f32)
            nc.sync.dma_start(Wc1s[:], Wc1_d[:])
            bc1 = pp.tile([128, 1], f32)
            nc.sync.dma_start(bc1[:], bc1_d[:])
            Wc2 = pp.tile([128, F], f32)
            nc.sync.dma_start(Wc2[:], Wc2_d[:])
            bc2 = pp.tile([F, 1], f32)
            nc.sync.dma_start(bc2[:], bc2_d[:])
            Wc3 = pp.tile([F, 1], f32)
            nc.sync.dma_start(Wc3[:], Wc3_d[:])
            bc3 = pp.tile([1, 1], f32)
            nc.sync.dma_start(bc3[:], bc3_d[:])

            with tc.tile_pool(name="conv", bufs=1) as cp:
                u_nm = cp.tile([128, NROUND, F], f32, tag="u_nm",
                               name="u_nm0", bufs=2)
                nc.sync.dma_start(
                    u_nm[:], u0b_d.ap().rearrange("(t p) f -> p t f", p=128))
                def coll_piece(dst, which):
                    src = (bounceA, bounceB, bounceC)[which]
                    base = (0, 26624, 39936)[which]
                    rows = (3328, 1664, 1664)[which]
                    nc.gpsimd.collective_compute(
                        "AllGather", mybir.AluOpType.bypass,
                        replica_groups=[list(range(NCORES))],
                        ins=[src.ap()],
                        outs=[dst.ap()[base:base + NCORES * rows, :]])

                # layer-0 table comes prebuilt from the host; collectives for
                # layer l+1 write the alternate table while layer l gathers.
                tabs = [u0full_d, u_alls[1], u_alls[0], u_alls[1]]
                for layer in range(4):
                    u_all = tabs[layer]

                    fo = F if layer < 3 else 1
                    hout = (hcatA[0:64, :] if layer == 0 else
                            hcatA[64:128, :] if layer == 1 else
                            hcatB[0:64, :] if layer == 2 else
                            hcatB[64:65, :])
                    if layer < 3:
                        u_next = cp.tile([128, NROUND, F], f32, tag="u_nm",
                                         name=f"u_nm{layer + 1}", bufs=2)
                    s_fm = cp.tile([F, SHARD], f32, tag="s_fm",
                                   name=f"s_fm{layer}", bufs=1)

                    collA_issued = 0
                    for (k0, k1, c0, c1) in chunks:
                        cols = c1 - c0
                        gt = cp.tile([128, CH_MAX, F], f32, tag="gt",
                                     name=f"gt{layer}_{k0}", bufs=2)
                        for c in range(c0, c1):
                            nc.gpsimd.indirect_dma_start(
                                out=gt[:, c - c0, :], out_offset=None,
                                in_=u_all[:],
                                in_offset=bass.IndirectOffsetOnAxis(
                                    ap=gidx[:, c:c + 1], axis=0))
                        for k in range(k0, k1):
                            dk = int(D[k])
                            q0 = int(callbase[k]) - c0
                            # in-place tree reduction of dk slab columns
                            d_cur = dk
                            while d_cur > 1:
                                h = d_cur // 2
                                nc.vector.tensor_tensor(
                                    out=gt[:, q0:q0 + h, :],
                                    in0=gt[:, q0:q0 + h, :],
                                    in1=gt[:, q0 + d_cur - h:q0 + d_cur, :],
                                    op=mybir.AluOpType.add)
                                d_cur -= h
                            s_nm = cp.tile([128, F], f32, tag="s_nm",
                                           name=f"s_nm{layer}_{k}", bufs=4)
                            if dk > 0:
                                nc.vector.tensor_tensor(
                                    out=s_nm[:], in0=gt[:, q0, :],
                                    in1=u_nm[:, k, :],
                                    op=mybir.AluOpType.add)
                            else:
                                nc.vector.tensor_copy(s_nm[:], u_nm[:, k, :])
                            nc.vector.tensor_tensor(
                                out=s_nm[:], in0=s_nm[:],
                                in1=dinv[:, k:k + 1].to_broadcast([128, F]),
                                op=mybir.AluOpType.mult)
                            # transpose to feature-major
                            ptile = ps_t.tile([F, 128], f32, tag="trf",
                                              name=f"ptf{layer}_{k}", bufs=2)
                            nc.tensor.matmul(ptile[:], s_nm[:], ident[:],
                                             is_transpose=True)
                            nc.scalar.activation(
                                s_fm[:, k * 128:(k + 1) * 128], ptile[:],
                                AF.Copy)
                            # transform + tanh + inline staging per 4 rounds
                            if k % 4 == 3 or k == NROUND - 1:
                                t0 = (k // 4) * 4
                                nc0 = t0 * 128
                                nc1 = (k + 1) * 128
                                pw = ps_w.tile([fo, 512], f32, tag="pw",
                                               name=f"pw{layer}_{t0}")
                                nc.tensor.matmul(pw[:, 0:nc1 - nc0],
                                                 Ws[layer][:],
                                                 s_fm[:, nc0:nc1],
                                                 start=True, stop=True)
                                nc.scalar.activation(
                                    hout[:, nc0:nc1], pw[:, 0:nc1 - nc0],
                                    AF.Tanh, bias=bs[layer][:])
                                if layer < 3:
                                    for t in range(t0, k + 1):
                                        pt2 = ps_t.tile(
                                            [128, F], f32, tag="trs",
                                            name=f"pts{layer}_{t}", bufs=2)
                                        ident64 = (ident[64:128, 64:128]
                                                   if layer == 1
                                                   else ident[0:F, 0:F])
                                        nc.tensor.matmul(
                                            pt2[:],
                                            hout[:, t * 128:(t + 1) * 128],
                                            ident64,
                                            is_transpose=True)
                                        nc.vector.tensor_tensor(
                                            out=u_next[:, t, :], in0=pt2[:],
                                            in1=dinv[:, t:t + 1]
                                            .to_broadcast([128, F]),
                                            op=mybir.AluOpType.mult)
                        if layer < 3 and collA_issued == 0 and k1 >= 28:
                            # rounds 0-27 staged: ship piece A of next table
                            # while the rest of this layer still gathers.
                            nc.sync.dma_start(
                                bounceA.ap().rearrange(
                                    "(t p) f -> p t f", p=128),
                                u_next[:, 0:26, :])
                            coll_piece(u_alls[(layer + 1) % 2], 0)
                            collA_issued = 1
                        if layer < 3 and collA_issued == 1 and k1 >= 40:
                            nc.sync.dma_start(
                                bounceB.ap().rearrange(
                                    "(t p) f -> p t f", p=128),
                                u_next[:, 26:39, :])
                            coll_piece(u_alls[(layer + 1) % 2], 1)
                            collA_issued = 2
                    if layer < 3:
                        nc.sync.dma_start(
                            bounceC.ap().rearrange("(t p) f -> p t f", p=128),
                            u_next[:, 39:NROUND, :])
                        coll_piece(u_alls[(layer + 1) % 2], 2)
                        u_nm = u_next

            # ================= sort-pool + classifier =================
            with tc.tile_pool(name="poolp", bufs=1) as qp:
                h3 = hcatB[64:65, :]
                nc.sync.dma_start(d6656.ap(), h3)
                h3gt = qp.tile([13, NPG], f32)
                nc.sync.dma_start(
                    h3gt[:],
                    d6656.ap()[:, 0:6500].rearrange(
                        "one (i g) -> (one g) i", g=13))
                h3g = h3gt[:]

                m8a = qp.tile([13, 8], f32)
                i8a = qp.tile([13, 8], mybir.dt.uint32)
                nc.vector.max(m8a[:], h3g)
                nc.vector.max_index(i8a[:], m8a[:], h3g)
                h3m = qp.tile([13, NPG], f32)
                nc.vector.match_replace(h3m[:], m8a[:], h3g, -2.0)
                m8b = qp.tile([13, 8], f32)
                i8b = qp.tile([13, 8], mybir.dt.uint32)
                nc.vector.max(m8b[:], h3m[:])
                nc.vector.max_index(i8b[:], m8b[:], h3m[:])

                idx2d = qp.tile([13, 16], f32)
                nc.vector.memset(idx2d[:], 0.0)
                nc.vector.tensor_copy(idx2d[:, 0:8], i8a[:])
                nc.vector.tensor_copy(idx2d[:, 8:15], i8b[:, 0:7])
                gof = qp.tile([13, 1], mybir.dt.int32)
                nc.gpsimd.iota(gof[:], [[0, 1]], base=0, channel_multiplier=1)
                goff = qp.tile([13, 1], f32)
                nc.vector.tensor_copy(goff[:], gof[:])
                # absolute slot = pos*13 + g
                nc.vector.tensor_scalar(
                    out=idx2d[:], in0=idx2d[:], scalar1=13.0,
                    scalar2=None, op0=mybir.AluOpType.mult)
                nc.vector.tensor_scalar(
                    out=idx2d[:], in0=idx2d[:], scalar1=goff[:, 0:1],
                    scalar2=None, op0=mybir.AluOpType.add)
                idx16 = qp.tile([13, 16], mybir.dt.int16)
                nc.vector.tensor_copy(idx16[:], idx2d[:])
                nc.sync.dma_start(
                    d208.ap().rearrange("one (g r) -> (one g) r", g=13),
                    idx16[:])
                idx128 = qp.tile([128, 13], mybir.dt.int16)
                for kk in range(8):
                    nc.sync.dma_start(
                        idx128[kk * 16:(kk + 1) * 16, :],
                        d208.ap().rearrange("one (s p) -> (one p) s", p=16))

                poolA = qp.tile([128, 208], f32)
                nc.gpsimd.ap_gather(poolA[:], hcatA[:], idx128[:],
                                    channels=128, num_elems=SHARD, d=1,
                                    num_idxs=208)
                poolB = qp.tile([80, 208], f32)
                nc.gpsimd.ap_gather(poolB[:], hcatB[:], idx128[0:80, :],
                                    channels=80, num_elems=SHARD, d=1,
                                    num_idxs=208)

                z1 = []
                for mh in range(2):
                    pz = ps_w.tile([128, 13], f32, tag="pw", name=f"pz{mh}")
                    first = True
                    for r in range(K_TOP):
                        nc.tensor.matmul(
                            pz[:], WA[:, r, mh * 128:(mh + 1) * 128],
                            poolA[:, r:r + 16 * 12 + 1:16],
                            start=first, stop=False)
                        first = False
                        nc.tensor.matmul(
                            pz[:], WB[0:65, r, mh * 128:(mh + 1) * 128],
                            poolB[0:65, r:r + 16 * 12 + 1:16],
                            start=False, stop=(r == K_TOP - 1))
                    zz = qp.tile([128, 13], f32, tag=f"z1_{mh}", name=f"z1_{mh}")
                    nc.scalar.activation(zz[:], pz[:], AF.Relu,
                                         bias=be[:, mh:mh + 1],
                                         scale=sc[:, mh:mh + 1])
                    z1.append(zz)
                pz2 = ps_w.tile([128, 13], f32, tag="pw", name="pz2")
                nc.tensor.matmul(pz2[:], Wc1s[:, 0, :], z1[0][:],
                                 start=True, stop=False)
                nc.tensor.matmul(pz2[:], Wc1s[:, 1, :], z1[1][:],
                                 start=False, stop=True)
                z2 = qp.tile([128, 13], f32)
                nc.scalar.activation(z2[:], pz2[:], AF.Relu, bias=bc1[:])
                pz3 = ps_w.tile([F, 13], f32, tag="pw", name="pz3")
                nc.tensor.matmul(pz3[:], Wc2[:], z2[:], start=True, stop=True)
                z3 = qp.tile([F, 13], f32)
                nc.scalar.activation(z3[:], pz3[:], AF.Relu, bias=bc2[:])
                pz4 = ps_w.tile([1, 13], f32, tag="pw", name="pz4")
                nc.tensor.matmul(pz4[:], Wc3[:], z3[:], start=True, stop=True)
                zf = qp.tile([1, 13], f32)
                nc.vector.tensor_scalar(out=zf[:], in0=pz4[:],
                                        scalar1=bc3[0:1, 0:1], scalar2=None,
                                        op0=mybir.AluOpType.add)
                nc.sync.dma_start(out_d[:], zf[:])

    nc.compile()
    return nc


def kernel(**inputs):
    from concourse import bass_utils

    x = np.asarray(inputs["x"], np.float32)
    edge_index = np.asarray(inputs["edge_index"])

    key = ("prog",)
    CALLS, D, callbase, gidxs, u0bs, dinv_nms, u0_full = _prep(x, edge_index)
    if key in _CACHE and _CACHE[key][0] == CALLS and np.array_equal(_CACHE[key][1], D):
        nc = _CACHE[key][2]
    else:
        nc = _build(CALLS, D, callbase)
        _CACHE[key] = (CALLS, D, nc)

    cw = _pack_classifier(inputs)
    in_maps = []
    for c in range(NCORES):
        m = {
            "u0b": u0bs[c],
            "u0_full": u0_full,
            "gidx": gidxs[c],
            "dinv_nm": dinv_nms[c],
            "WA": cw["WA"], "WB": cw["WB"], "sc": cw["sc"], "be": cw["be"],
            "Wc1s": cw["Wc1s"], "bc1": cw["bc1"], "Wc2": cw["Wc2"],
            "bc2": cw["bc2"], "Wc3": cw["Wc3"], "bc3": cw["bc3"],
        }
        for i in range(4):
            m[f"W{i}"] = np.asarray(inputs[f"W{i}"], np.float32).reshape(
                F, F if i < 3 else 1)
            m[f"b{i}"] = np.asarray(inputs[f"b{i}"], np.float32).reshape(
                F if i < 3 else 1, 1)
        in_maps.append(m)

    trace = os.environ.get("KERNEL_TRACE", "0") == "1"
    kwargs = {}
    if trace:
        import sys, types
        if "antenv.axon_hooks" not in sys.modules:
            sys.path.insert(0, "/root/.axon_site")
            from trn_agent_boot.trn_boot import _ntff_profile_via_ctypes
            mm = types.ModuleType("antenv.axon_hooks")
            mm.get_axon_ntff_profile_hook = (
                lambda: _ntff_profile_via_ctypes("/opt/axon/libaxon_pjrt.so"))
            sys.modules["antenv.axon_hooks"] = mm
        import tempfile
        kwargs = dict(trace=True, tmpdir=tempfile.mkdtemp())

    res = bass_utils.run_bass_kernel_spmd(
        nc, in_maps, core_ids=list(range(NCORES)), **kwargs)

    global LAST_EXEC_NS
    LAST_EXEC_NS = res.exec_time_ns

    out = np.zeros((G, 1), np.float32)
    for c in range(NCORES):
        ngr = GRAPHS_PER_CORE[c]
        out[GSTART[c]:GSTART[c] + ngr, 0] = res.results[c]["out"][0, :ngr]
    return out


LAST_EXEC_NS = None
